# revision 13
# baseline (speedup 1.0000x reference)
"""Trainium2 Bass kernel for a decoder self-attention layer (+residual).

Reference computation (fp32):
    q = inputs @ Wq.T ; k = inputs @ Wk.T ; v = inputs @ Wv.T   (biases are 0)
    per (batch, head):  attn = softmax(q k^T / sqrt(d_model)) v
    return inputs + attn

Shapes: inputs [S=2048, B=4, D=1024], W* [1024, 1024], 16 heads x 64 dims.
The mask is all-False and biases are all-zero by the input spec, so neither is
applied on device.

Sharding: tensor-parallel over heads. Core c owns heads {2c, 2c+1} = rows
[128c, 128c+128) of Wq/Wk/Wv and feature columns [128c, 128c+128) of the
output. Every core reads the full `inputs`; the host concatenates the
per-core outputs along the feature axis.

Per-core data flow (fp8e4 DoubleRow matmuls where the contraction allows,
bf16 elsewhere, accumulation fp32):
  1. X^T into SBUF as fp8e4: SWDGE DMAs load token-tiles natural-layout with
     an inline fp32->bf16 cast; the PE transposes 128x128 blocks via identity
     matmuls (8 per PSUM bank, drained by one batched DVE copy that also
     casts bf16->fp8). No hardware DMA-transposes anywhere (xbar-mode hazard,
     ~100us dead time). W^T is produced the same way but scaled x16 on the
     drain (ScalarE Copy, scale=16) so fp8 quantization of the ~N(0,1/32)
     weights stays out of the denormal range.
  2. Q^T, K^T feature-major via fp8 DoubleRow W^T-stationary matmuls (K=256
     per instruction: d-block pairs), drained to bf16; V token-major via PE
     transpose of V^T, drained to fp8 with a fused ones-column (=16.0 to
     match V's x16 scale) for the softmax denominator.
  3. Per sweep (batch, 512 queries): scores S^T = K Q^T in bf16 with the two
     heads row-packed on the PE (K=64 at partition bases 0/64); exp() on
     ScalarE straight from PSUM with the 1/(32*256) scale folded in (Q and K
     each carry x16 from the weight scale), emitting fp8e4 P^T directly into
     a per-sweep [128, 16, 1024] tile.
  4. O = P V via fp8 DoubleRow with P^T chunk-pairs as the stationary operand
     (K=256: two 128-key chunks per instruction); column 64 of the moving
     operand (V|16) accumulates the softmax denominator r (x16, cancels).
     Each accumulation group's 8 pair-matmuls are emitted contiguously; the
     previous sweep's PV groups are interleaved between the current sweep's
     score/exp quarters to keep both PE and ScalarE busy.
  5. Finalize on VectorE: out = (O * 1/r) + x_residual, fp32.

fp8 error budget: X,P,V at ~3% element error and W at ~3% give the attention
output ~6% relative error, but the attention output is ~0.02 in magnitude vs
the residual-dominated output scale of ~5, so the end-to-end relative error
stays ~3e-4 (measured), well under the 2e-2 gate.
"""

import os
import sys

sys.path.insert(0, "/opt/trn_rl_repo")

# The kernel executes via bass2jax on the axon-tunneled NeuronCores; a
# CPU-pinned JAX_PLATFORMS (sometimes set for reference-side jax) would hide
# them. Only drop the pin if jax has not been imported yet.
if "jax" not in sys.modules and os.environ.get("JAX_PLATFORMS") == "cpu":
    os.environ.pop("JAX_PLATFORMS")

import numpy as np

import concourse.bass as bass
import concourse.tile as tile
from concourse import bacc, mybir
from concourse import bass_utils

S, B, D = 2048, 4, 1024
NH, DH = 16, 64
NCORES = 8
DCOL = D // NCORES  # 128 projection dims (2 heads) per core
NSQH = 4  # 512-query sweeps per batch
NKT = S // 128  # 16 key chunks per sweep
BF16 = mybir.dt.bfloat16
F32 = mybir.dt.float32
FP8 = mybir.dt.float8e4
AF = mybir.ActivationFunctionType
ALU = mybir.AluOpType
PM = mybir.MatmulPerfMode

WSCALE = 16.0  # fp8 weight pre-scale; folded out via exp scale / denominator
EXP_SCALE = 1.0 / (32.0 * WSCALE * WSCALE)


def attention_kernel(tc, x, xres, wq, wk, wv, out):
    nc = tc.nc
    with (
        tc.tile_pool(name="persist", bufs=1) as persist,
        tc.tile_pool(name="xnat", bufs=3) as xnat_pool,
        tc.tile_pool(name="xt", bufs=2) as xt_pool,
        tc.tile_pool(name="qkv", bufs=2) as qkv_pool,
        tc.tile_pool(name="vstage", bufs=2) as vstage_pool,
        tc.tile_pool(name="pt", bufs=2) as pt_pool,
        tc.tile_pool(name="io", bufs=2) as io_pool,
        tc.tile_pool(name="small", bufs=4) as small_pool,
        tc.tile_pool(name="psQ", bufs=2, space="PSUM") as psQ,  # qkv & transposes
        tc.tile_pool(name="psS", bufs=2, space="PSUM") as psS,  # scores (2x2 banks)
        tc.tile_pool(name="psO", bufs=1, space="PSUM") as psO,  # O accum (2 banks)
    ):
        ident = persist.tile([128, 128], BF16, tag="ident")
        wt_q = persist.tile([128, D // 128, 128], FP8, tag="wt_q")
        wt_k = persist.tile([128, D // 128, 128], FP8, tag="wt_k")
        wt_v = persist.tile([128, D // 128, 128], FP8, tag="wt_v")

        from concourse.masks import make_identity

        make_identity(nc, ident[:])

        # All transposes happen on the PE (in_.T via identity matmul): the
        # hardware DMA-transpose path is avoided entirely because any
        # transpose-DMA serializes globally against every copy-DMA (xbar-mode
        # hazard). The fp32->bf16 cast happens inline in the SWDGE load;
        # the bf16->fp8 cast happens in the PSUM drain.
        # Eight 128x128 transposes fill one PSUM bank exactly (each is a
        # single overwriting matmul group, so the whole-bank has_written
        # clear on start is harmless) and drain with ONE batched copy.
        def pe_transposeN(src_nat, blks, out_ap, wscale=False):
            pxt = psQ.tile([128, len(blks), 128], BF16, tag="q2", name="pxt")
            for q, blk in enumerate(blks):
                nc.tensor.transpose(
                    pxt[:, q, :], src_nat[:, blk * 128 : (blk + 1) * 128], ident[:]
                )
            if wscale:
                # weights: scale x16 on the drain so fp8 stays out of denormals
                nc.scalar.mul(out_ap, pxt[:], WSCALE)
            else:
                nc.vector.tensor_copy(out_ap, pxt[:])

        for w_ap, wt in ((wq, wt_q), (wk, wt_k), (wv, wt_v)):
            wn = xnat_pool.tile([128, D], BF16, tag="xn", name="wn")
            nc.gpsimd.dma_start(wn[:], w_ap)  # cast fp32 -> bf16 inline
            pe_transposeN(wn, range(8), wt[:, :, :], wscale=True)

        def emit_phase1_tiles(b, xt_b, tis, fast=False):
            # fast path (prologue only): SWDGE cast-descriptor generation on
            # the Pool engine is serial (~2us/tile) and gates batch 0's X^T;
            # instead load fp32 over the HWDGE ring and cast on the
            # prologue-idle VectorE.
            for ti in tis:
                xn = xnat_pool.tile([128, D], BF16, tag="xn", name="xn")
                if fast:
                    xf = xnat_pool.tile([128, D], F32, tag="xf", name="xf")
                    nc.sync.dma_start(xf[:], x[ti * 128 : (ti + 1) * 128, b, :])
                    nc.vector.tensor_copy(xn[:], xf[:])
                else:
                    nc.gpsimd.dma_start(xn[:], x[ti * 128 : (ti + 1) * 128, b, :])
                pe_transposeN(xn, range(8), xt_b[:, :, ti * 128 : (ti + 1) * 128])

        # QKV projections: fp8 DoubleRow, contracting d-block PAIRS (K=256
        # per instruction) -> 4 matmuls per 512-token chunk instead of 8.
        def emit_phase2_ti(xt_b, ti, qt_b, kt_b, v1_b):
            for wt, dst in ((wt_q, qt_b), (wt_k, kt_b)):
                pqk = psQ.tile([128, 512], F32, tag="q2", name="pqk")
                for p in range(D // 256):
                    nc.tensor.matmul(
                        pqk[:],
                        wt[:, 2 * p : 2 * p + 2, :],
                        xt_b[:, 2 * p : 2 * p + 2, ti * 512 : (ti + 1) * 512],
                        start=(p == 0),
                        stop=(p == D // 256 - 1),
                        perf_mode=PM.DoubleRow,
                    )
                nc.vector.tensor_copy(dst[:, ti * 512 : (ti + 1) * 512], pqk[:])
            pv = psQ.tile([128, 512], F32, tag="q2", name="pv")
            for p in range(D // 256):
                nc.tensor.matmul(
                    pv[:],
                    wt_v[:, 2 * p : 2 * p + 2, :],
                    xt_b[:, 2 * p : 2 * p + 2, ti * 512 : (ti + 1) * 512],
                    start=(p == 0),
                    stop=(p == D // 256 - 1),
                    perf_mode=PM.DoubleRow,
                )
            vstage = vstage_pool.tile([128, 512], BF16, tag="vstage")
            nc.vector.tensor_copy(vstage[:], pv[:])
            pvt = psQ.tile([128, 4, 128], BF16, tag="q2", name="pvt")
            for tt in range(4):
                nc.tensor.transpose(
                    pvt[:, tt, :], vstage[:, tt * 128 : (tt + 1) * 128], ident[:]
                )
            nc.vector.tensor_copy(
                v1_b[:, ti * 4 : (ti + 1) * 4, :, 0:64],
                pvt.rearrange("p t (lh dh) -> p t lh dh", lh=2),
            )

        def alloc_qkv():
            qt_b = qkv_pool.tile([128, S], FP8, tag="qt_b", name="qt_b")
            kt_b = qkv_pool.tile([128, S], FP8, tag="kt_b", name="kt_b")
            v1_b = qkv_pool.tile([128, NKT, 2, 65], FP8, tag="v1_b", name="v1_b")
            nc.vector.memset(v1_b[:, :, :, 64:65], WSCALE)
            return qt_b, kt_b, v1_b

# (per-batch QKV is staggered into the previous batch's sweeps; see below)

        class Sweep:
            __slots__ = ("b", "sqh", "pt", "xres_t", "v1_b", "o_ps", "ostage")

        # scores in fp8 with DoublePixel: the PE streams two moving columns
        # per cycle (validated on HW), halving the score stream time, which
        # is the largest single PE cost. Q/K quantization adds ~1% to P.
        def emit_scores_quarter(sw, quarter, qt_b, kt_b):
            for kt_i in range(quarter * 4, quarter * 4 + 4):
                s_ps = psS.tile([128, 1024], F32, tag="s_ps")
                for lh in range(2):
                    nc.tensor.matmul(
                        s_ps[:, lh * 512 : (lh + 1) * 512],
                        kt_b[lh * 64 : (lh + 1) * 64, kt_i * 128 : (kt_i + 1) * 128],
                        qt_b[
                            lh * 64 : (lh + 1) * 64,
                            sw.sqh * 512 : (sw.sqh + 1) * 512,
                        ],
                        perf_mode=PM.DoublePixel,
                    )
                nc.scalar.activation(
                    sw.pt[:, kt_i, :], s_ps[:], AF.Exp, scale=EXP_SCALE
                )

        # O = P V: fp8 DoubleRow with P^T chunk-PAIRS stationary (K=256);
        # 8 pair-matmuls per accumulation group, emitted contiguously.
        def emit_pv_quarter(sw, quarter):
            if quarter == 0:
                sw.o_ps = psO.tile([128, 8, 128], F32, tag="o_ps")
            for g in (2 * quarter, 2 * quarter + 1):
                lh, j = g // 4, g % 4
                for t in range(NKT // 2):
                    nc.tensor.matmul(
                        sw.o_ps[:, g, 0:65],
                        sw.pt[
                            :,
                            2 * t : 2 * t + 2,
                            lh * 512 + j * 128 : lh * 512 + (j + 1) * 128,
                        ],
                        sw.v1_b[:, 2 * t : 2 * t + 2, lh, :],
                        start=(t == 0),
                        stop=(t == NKT // 2 - 1),
                        perf_mode=PM.DoubleRow,
                    )

        def emit_finalize(sw):
            rinv = small_pool.tile([128, 8], F32, tag="rinv")
            nc.vector.reciprocal(rinv[:], sw.o_ps[:, :, 64])
            sw.ostage = io_pool.tile([128, 4, DCOL], F32, tag="ostage")
            for g in range(8):
                lh, j = g // 4, g % 4
                nc.vector.scalar_tensor_tensor(
                    out=sw.ostage[:, j, lh * 64 : (lh + 1) * 64],
                    in0=sw.o_ps[:, g, 0:64],
                    scalar=rinv[:, g : g + 1],
                    in1=sw.xres_t[:, j, lh * 64 : (lh + 1) * 64],
                    op0=ALU.mult,
                    op1=ALU.add,
                )
            nc.gpsimd.dma_start(
                out[sw.sqh * 512 : (sw.sqh + 1) * 512, sw.b, :].rearrange(
                    "(j p) d -> p j d", p=128
                ),
                sw.ostage[:],
            )

        prev = None
        # batch 0's X^T is the prologue: interleave its QKV per 4-tile group
        # so the first score matmuls (which only need Q/K for the first 512
        # tokens) fire after ~1/4 of the loads instead of all of them. Later
        # batches' X^T production is spread across the previous batch's
        # sweeps (one token-tile per quarter), so nothing clusters at batch
        # boundaries.
        xt_b = xt_pool.tile([128, D // 128, S], FP8, tag="xt_b", name="xt_b")
        qkv0 = alloc_qkv()
        for g in range(4):
            emit_phase1_tiles(0, xt_b, range(g * 4, (g + 1) * 4), fast=True)
            emit_phase2_ti(xt_b, g, *qkv0)
        qkv_cur = qkv0
        for b in range(B):
            qt_b, kt_b, v1_b = qkv_cur
            xt_next = None
            qkv_next = None
            if b + 1 < B:
                xt_next = xt_pool.tile([128, D // 128, S], FP8, tag="xt_b", name="xt_b")
                qkv_next = alloc_qkv()
            for sqh in range(NSQH):
                sw = Sweep()
                sw.b, sw.sqh, sw.v1_b = b, sqh, v1_b
                sw.pt = pt_pool.tile([128, NKT, 1024], FP8, tag="pt", name="pt")
                sw.xres_t = io_pool.tile([128, 4, DCOL], F32, tag="xres")
                nc.gpsimd.dma_start(
                    sw.xres_t[:],
                    xres[sqh * 512 : (sqh + 1) * 512, b, :].rearrange(
                        "(j p) d -> p j d", p=128
                    ),
                )
                for quarter in range(4):
                    emit_scores_quarter(sw, quarter, qt_b, kt_b)
                    if prev is not None:
                        emit_pv_quarter(prev, quarter)
                    if xt_next is not None:
                        emit_phase1_tiles(b + 1, xt_next, [sqh * 4 + quarter])
                # finalize first: its DVE reciprocal/STT must not queue behind
                # the staggered QKV's PSUM drains on the in-order DVE queue
                # (psO is single-buffered; the next sweep's PV waits on it).
                if prev is not None:
                    emit_finalize(prev)
                # next batch's QKV for 512-token chunk sqh: its X^T tiles
                # (4*sqh .. 4*sqh+3) were just produced in this sweep's
                # quarters, so by sweep 3 the next batch's Q/K/V are complete
                # and its first scores can fire at the batch boundary with no
                # ScalarE gap.
                if qkv_next is not None:
                    emit_phase2_ti(xt_next, sqh, *qkv_next)
                prev = sw
            xt_b = xt_next
            qkv_cur = qkv_next
        for quarter in range(4):
            emit_pv_quarter(prev, quarter)
        emit_finalize(prev)


_CACHED = None


def _build():
    global _CACHED
    if _CACHED is not None:
        return _CACHED
    nc = bacc.Bacc("TRN2", target_bir_lowering=False, debug=False, num_devices=NCORES)
    x = nc.dram_tensor("x", [S, B, D], F32, kind="ExternalInput").ap()
    xres = nc.dram_tensor("xres", [S, B, DCOL], F32, kind="ExternalInput").ap()
    wq = nc.dram_tensor("wq", [DCOL, D], F32, kind="ExternalInput").ap()
    wk = nc.dram_tensor("wk", [DCOL, D], F32, kind="ExternalInput").ap()
    wv = nc.dram_tensor("wv", [DCOL, D], F32, kind="ExternalInput").ap()
    out = nc.dram_tensor("out", [S, B, DCOL], F32, kind="ExternalOutput").ap()
    with tile.TileContext(nc) as tc:
        attention_kernel(tc, x, xres, wq, wk, wv, out)
    nc.compile()
    _CACHED = nc
    return nc


def make_in_maps(inputs, Wq, Wk, Wv):
    x = np.ascontiguousarray(inputs, dtype=np.float32)
    maps = []
    for c in range(NCORES):
        sl = slice(c * DCOL, (c + 1) * DCOL)
        maps.append(
            {
                "x": x,
                "xres": np.ascontiguousarray(x[:, :, sl]),
                "wq": np.ascontiguousarray(Wq[sl], dtype=np.float32),
                "wk": np.ascontiguousarray(Wk[sl], dtype=np.float32),
                "wv": np.ascontiguousarray(Wv[sl], dtype=np.float32),
            }
        )
    return maps


def run(inputs, Wq, Wk, Wv, **run_kwargs):
    nc = _build()
    in_maps = make_in_maps(inputs, Wq, Wk, Wv)
    res = bass_utils.run_bass_kernel_spmd(
        nc, in_maps, core_ids=list(range(NCORES)), **run_kwargs
    )
    full = np.concatenate([res.results[c]["out"] for c in range(NCORES)], axis=2)
    return np.ascontiguousarray(full, dtype=np.float32), res


def kernel(inputs, mask, Wq, bq, Wk, bk, Wv, bv):
    # mask is all-False and biases are zero by the problem's input spec; they
    # do not alter the result and are not applied.
    out, _ = run(np.asarray(inputs), np.asarray(Wq), np.asarray(Wk), np.asarray(Wv))
    return out
